# revision 31
# baseline (speedup 1.0000x reference)
"""Trainium2 Bass kernel for EvalHead (NMS detection decode).

Computes, for x [B=16, C=15, H=512, W=512] fp32:
  scores = x[:,0]; peak = (scores > 0.5) & (scores == maxpool3x3(scores))
  out[b,h,w,:] = [score, cx-hx, cy-hy, cx+hx, cy+hy, lm0x+px, lm0y+py, ...] * peak
  where cx = px + x[:,1], cy = py + x[:,2], hx = 0.5*x[:,3], hy = 0.5*x[:,4],
        px = 4*w+2, py = 4*h+2.
Output: [16, 512, 512, 15] fp32.

Sharding: pure data parallel over batch — 2 images per core across 8 cores.
Per-core layout: partition = image row; 4 tiles of [128 rows, 512 cols] per
image. Vertical pool via +-1-row shifted HBM loads (edge rows clamped, since
max(a,a,b)=max(a,b) matches SAME padding); horizontal pool via shifted
free-dim slices of an edge-duplicated padded tile. Output assembled in SBUF
channel-interleaved [128, 512*15] so the store is one contiguous DMA.
"""

import numpy as np

B = 16
N_CORES = 8
B_LOCAL = B // N_CORES  # 2 images per core
C = 15
H = 512
W = 512
PT = 128                 # partition tile height (rows)
NT = H // PT             # 4 row-tiles per image
STRIDE = 4
OFF_Y = 2.0
OFF_X = 2.0
THRESHOLD = 0.5
NEG = -1e30

_CACHE = {}


def _build_nc(loop_k: int = 1, sup_mode: str = "v3"):
    """Build the per-core Bass module. loop_k > 1 wraps the whole body in a
    hardware For loop (used only for timing measurements). sup_mode selects
    tuning variants; 'v3' (default/production): input loads on the ACT HWDGE
    ring + output stores on the SP ring (both stream in parallel, HBM-bound),
    split channel loads, sup/sdn shifted loads on the SWDGE ring, 3-deep
    input buffering, decode written straight into the interleaved output
    tile with in-place masking."""
    from contextlib import ExitStack, nullcontext

    import bass_rust
    import concourse.tile as tile
    from concourse import bacc, mybir
    from concourse.alu_op_type import AluOpType

    f32 = mybir.dt.float32
    Act = bass_rust.ActivationFunctionType

    nc = bacc.Bacc(None, target_bir_lowering=False)

    x = nc.dram_tensor("x", [B_LOCAL, C, H, W], f32, kind="ExternalInput")
    pxd = nc.dram_tensor("pxd", [PT, W], f32, kind="ExternalInput")
    pyd = nc.dram_tensor("pyd", [NT, PT], f32, kind="ExternalInput")
    out = nc.dram_tensor("out", [B_LOCAL, H, W, C], f32, kind="ExternalOutput")

    with tile.TileContext(nc) as tc, ExitStack() as ctx:
        loop = tc.For_i(0, loop_k, 1) if loop_k > 1 else nullcontext()
        ctx.enter_context(loop)
        in_bufs = 3 if sup_mode in ("v2", "v3", "v4") else 2
        const = ctx.enter_context(tc.tile_pool(name="const", bufs=1))
        inp = ctx.enter_context(tc.tile_pool(name="inp", bufs=in_bufs))
        sp = ctx.enter_context(tc.tile_pool(name="sp", bufs=2))
        mid = ctx.enter_context(tc.tile_pool(name="mid", bufs=2))
        outp = ctx.enter_context(tc.tile_pool(name="outp", bufs=2))

        out_flat = out.rearrange("b h w c -> b h (w c)")
        pxt = const.tile([PT, W], f32)
        nc.sync.dma_start(pxt[:], pxd[:])
        pyt = const.tile([PT, NT], f32)
        nc.sync.dma_start(pyt[:], pyd.rearrange("t p -> p t"))
        # px broadcast views: [p][j][w] with j (landmark idx) as a 0-step dim
        pxb = pxt[:].broadcast_to([PT, W, 5]).rearrange("p w j -> p j w")

        v4 = sup_mode == "v4"

        def emit_decode(b, t, r0, v15, smid, m, mb):
            pycol = pyt[:, t:t + 1]
            cxp = mid.tile([PT, W], f32)
            nc.gpsimd.tensor_tensor(cxp[:], v15[:, 1, :], pxt[:], op=AluOpType.add)
            cyp = mid.tile([PT, W], f32)
            if v4:
                # DVE tensor_scalar runs fp32 at 2x; cheaper than ACT here
                nc.vector.tensor_scalar(cyp[:], v15[:, 2, :], pycol, None, AluOpType.add)
            else:
                nc.scalar.activation(cyp[:], v15[:, 2, :], Act.Identity, bias=pycol, scale=1.0)

            # decode straight into the interleaved output tile, mask in place
            ot = outp.tile([PT, W * C], f32)
            ot4 = ot.rearrange("p (w c) -> p w c", c=C)
            nc.vector.scalar_tensor_tensor(
                ot4[:, :, 1], v15[:, 3, :], -0.5, cxp[:], AluOpType.mult, AluOpType.add)
            nc.vector.scalar_tensor_tensor(
                ot4[:, :, 3], v15[:, 3, :], 0.5, cxp[:], AluOpType.mult, AluOpType.add)
            nc.vector.scalar_tensor_tensor(
                ot4[:, :, 2], v15[:, 4, :], -0.5, cyp[:], AluOpType.mult, AluOpType.add)
            nc.vector.scalar_tensor_tensor(
                ot4[:, :, 4], v15[:, 4, :], 0.5, cyp[:], AluOpType.mult, AluOpType.add)

            # landmarks: channels 5..14 = 5 (x, y) pairs
            lmp = v15[:, 5:C, :].rearrange("p (j k) w -> p j k w", k=2)
            olm = ot4[:, :, 5:C].rearrange("p w (j k) -> p w j k", k=2)
            olx = olm[:, :, :, 0].rearrange("p w j -> p j w")
            oly = olm[:, :, :, 1].rearrange("p w j -> p j w")
            nc.gpsimd.tensor_tensor(olx, lmp[:, :, 0, :], pxb, op=AluOpType.add)
            nc.scalar.activation(oly, lmp[:, :, 1, :], Act.Identity, bias=pycol, scale=1.0)

            # ---- masking ----
            nc.vector.tensor_tensor(ot4[:, :, 0], smid, m[:], op=AluOpType.mult)
            mb4 = m[:].broadcast_to([PT, W, 4])
            bbox_eng = nc.gpsimd if v4 else nc.vector
            bbox_eng.tensor_tensor(ot4[:, :, 1:5], ot4[:, :, 1:5], mb4, op=AluOpType.mult)
            nc.vector.tensor_tensor(olx, olx, mb, op=AluOpType.mult)
            nc.vector.tensor_tensor(oly, oly, mb, op=AluOpType.mult)

            if v4:
                nc.sync.dma_start(out_flat[b, r0:r0 + PT, :], ot[:])
            else:
                nc.sync.dma_start(out[b, r0:r0 + PT, :, :], ot4[:, :, :])

        for b in range(B_LOCAL):
            for t in range(NT):
                r0 = PT * t

                # DMA ring split: input loads on the ACT HWDGE ring, output
                # store on the SP ring, so the two FIFOs stream in parallel
                # and HBM bandwidth (not one ring) is the binding limit.
                ldq = nc.scalar if sup_mode in ("split", "gp", "v2", "v3", "v4", "probe_io", "probe_nopool") else nc.sync
                in15 = inp.tile([PT, C * W], f32)
                v15 = in15.rearrange("p (c w) -> p c w", c=C)
                if sup_mode in ("v3", "v4"):
                    # split load: mask+bbox channels land first so decode
                    # starts earlier; landmark channels follow
                    ldq.dma_start(v15[:, 0:5, :], x[b, 0:5, r0:r0 + PT, :].rearrange("c p w -> p c w"))
                    ldq.dma_start(v15[:, 5:C, :], x[b, 5:C, r0:r0 + PT, :].rearrange("c p w -> p c w"))
                else:
                    ldq.dma_start(v15[:, :, :], x[b, :, r0:r0 + PT, :].rearrange("c p w -> p c w"))

                if sup_mode == "probe_io":
                    # bandwidth probe: store the input tile back as "output"
                    nc.sync.dma_start(out[b, r0:r0 + PT, :, :],
                                      in15.rearrange("p (w c) -> p w c", c=C))
                    continue

                smid = v15[:, 0, :]
                if sup_mode == "probe_nopool":
                    m = mid.tile([PT, W], f32)
                    nc.vector.tensor_scalar(m[:], smid, THRESHOLD, None, AluOpType.is_gt)
                    mb = m[:].broadcast_to([PT, W, 5]).rearrange("p w j -> p j w")
                    emit_decode(b, t, r0, v15, smid, m, mb)
                    continue
                # +-1-row shifted score tiles: bulk via on-chip partition-shifted
                # copy of the score channel (ACT HWDGE ring, off the SP ring that
                # carries the bulk HBM traffic); boundary row from HBM, clamped
                # at the image edge (max(a,a,b)==max(a,b) == SAME padding).
                sup = sp.tile([PT, W], f32)
                sdn = sp.tile([PT, W], f32)
                if sup_mode == "gp":
                    # partition-shifted copies on GpSimd (the cross-partition
                    # engine); only the boundary row comes from HBM.
                    nc.gpsimd.tensor_copy(sup[1:PT, :], v15[0:PT - 1, 0, :])
                    ldq.dma_start(sup[0:1, :], x[b, 0, max(r0 - 1, 0):max(r0 - 1, 0) + 1, :])
                    nc.gpsimd.tensor_copy(sdn[0:PT - 1, :], v15[1:PT, 0, :])
                    rdn = min(r0 + PT, H - 1)
                    ldq.dma_start(sdn[PT - 1:PT, :], x[b, 0, rdn:rdn + 1, :])
                elif sup_mode == "sbuf":
                    nc.scalar.dma_start(sup[1:PT, :], v15[0:PT - 1, 0, :])
                    nc.scalar.dma_start(sup[0:1, :], x[b, 0, max(r0 - 1, 0):max(r0 - 1, 0) + 1, :])
                    nc.scalar.dma_start(sdn[0:PT - 1, :], v15[1:PT, 0, :])
                    rdn = min(r0 + PT, H - 1)
                    nc.scalar.dma_start(sdn[PT - 1:PT, :], x[b, 0, rdn:rdn + 1, :])
                else:
                    sq = nc.gpsimd if sup_mode in ("v2", "v3", "v4") else ldq
                    if t > 0:
                        sq.dma_start(sup[:], x[b, 0, r0 - 1:r0 + PT - 1, :])
                    else:
                        sq.dma_start(sup[0:1, :], x[b, 0, 0:1, :])
                        sq.dma_start(sup[1:PT, :], x[b, 0, 0:PT - 1, :])
                    if t < NT - 1:
                        sq.dma_start(sdn[:], x[b, 0, r0 + 1:r0 + PT + 1, :])
                    else:
                        sq.dma_start(sdn[0:PT - 1, :], x[b, 0, r0 + 1:H, :])
                        sq.dma_start(sdn[PT - 1:PT, :], x[b, 0, H - 1:H, :])

                # ---- 3x3 max pool -> peak mask m ----
                # v1 is a rolling scratch: vmax partial, then hmax partial,
                # then the equality mask (WAW deps keep the order correct).
                v1 = mid.tile([PT, W], f32)
                nc.vector.tensor_tensor(v1[:], sup[:], sdn[:], op=AluOpType.max)
                vp = mid.tile([PT, W + 2], f32)
                nc.vector.tensor_tensor(vp[:, 1:W + 1], v1[:], smid, op=AluOpType.max)
                # duplicate-edge pad: max(v0,v0,v1) == max(v0,v1) == SAME pooling
                pad_eng = nc.scalar if sup_mode == "v4" else nc.vector
                if sup_mode == "v4":
                    pad_eng.copy(vp[:, 0:1], vp[:, 1:2])
                    pad_eng.copy(vp[:, W + 1:W + 2], vp[:, W:W + 1])
                else:
                    pad_eng.tensor_copy(vp[:, 0:1], vp[:, 1:2])
                    pad_eng.tensor_copy(vp[:, W + 1:W + 2], vp[:, W:W + 1])
                nc.vector.tensor_tensor(v1[:], vp[:, 0:W], vp[:, 1:W + 1], op=AluOpType.max)
                pooled = mid.tile([PT, W], f32)
                nc.vector.tensor_tensor(pooled[:], v1[:], vp[:, 2:W + 2], op=AluOpType.max)
                eq_eng = nc.vector
                eq_eng.tensor_tensor(v1[:], smid, pooled[:], op=AluOpType.is_equal)
                m = mid.tile([PT, W], f32)
                nc.vector.scalar_tensor_tensor(
                    m[:], smid, THRESHOLD, v1[:], AluOpType.is_gt, AluOpType.mult)
                mb = m[:].broadcast_to([PT, W, 5]).rearrange("p w j -> p j w")

                emit_decode(b, t, r0, v15, smid, m, mb)

    nc.compile()
    return nc


def _aux_inputs():
    pxd = (np.arange(W, dtype=np.float32) * STRIDE + OFF_X)[None, :].repeat(PT, 0)
    pyd = (np.arange(H, dtype=np.float32) * STRIDE + OFF_Y).reshape(NT, PT)
    return np.ascontiguousarray(pxd), np.ascontiguousarray(pyd)


def kernel(x: np.ndarray) -> np.ndarray:
    from concourse.bass_utils import run_bass_kernel_spmd

    if "nc" not in _CACHE:
        _CACHE["nc"] = _build_nc()
    nc = _CACHE["nc"]

    x = np.ascontiguousarray(np.asarray(x, dtype=np.float32))
    assert x.shape == (B, C, H, W), x.shape
    pxd, pyd = _aux_inputs()
    in_maps = [
        {"x": np.ascontiguousarray(x[i * B_LOCAL:(i + 1) * B_LOCAL]), "pxd": pxd, "pyd": pyd}
        for i in range(N_CORES)
    ]
    res = run_bass_kernel_spmd(nc, in_maps, list(range(N_CORES)))
    return np.concatenate([res.results[i]["out"] for i in range(N_CORES)], axis=0)
